# revision 6
# baseline (speedup 1.0000x reference)
"""Trainium2 Bass kernel for a single-layer bigram language model
(embed + 16-head causal attention + vocab lm_head).

Sharding: 8 cores = 4 batches x 2 vocab halves. Core c handles batch c//2
and vocab columns [(c%2)*16000, (c%2+1)*16000). Attention is replicated
across the 2 cores of a batch pair (cheap: ~15% of FLOPs); the lm_head
(dominant cost) is fully sharded. No collectives needed.

v2: the lm_head runs in fp8 (e4m3) DoubleRow perf mode (2 contraction
rows per PE cycle => 2x bf16 throughput, 4x under the TimelineSim cost
model). Both operands are scaled by 2^11 before quantization; the
product scale 2^-22 plus the bias add are folded into one Activation
op. Accuracy: the max-abs error of full-fp8 logits is driven entirely
by the first ~128 tokens (softmax over few positions => large attention
out rows). Fix: an fp8 *residual* of the attention out for token tile 0
(same 2^11 scale, so it accumulates into the same PSUM group) adds a
second-order correction only where needed: measured rel err 1.6e-2 vs
2.2e-2 uncorrected (gate: 2e-2).

Phase-4 layout is [vocab_partition, token_free] (the transpose of v1):
  - bias becomes per-partition => fused into the ACT descale op
    (no DVE work in phase 4 at all)
  - logits DMA out as [VSH, T]; the host transposes when assembling.

All other matmuls run as float32r (fp32 data, PE truncates to fp22)
which runs at full bf16 speed for moving dims >= 256, with fp32 PSUM
accumulation.

Layout strategy: everything downstream of the embedding is kept
transposed ([feature, token]) so all matmul contractions have their K
dim on partitions:
  xT[e, t]  = (tok_emb[idx] + pos_emb).T        (PE-transposed after gather)
  qT/kT     = Wq_pair.T @ xT  (pairs of heads stacked on partitions 0:64 / 64:128)
  scoresT   = kT_h.T-slices @ qT_h  -> [s, t] tiles; exp on ACT; causal mask on DVE
  pv        = [v_h | 1].T @ e       -> rows 0:64 = unnormalized outT, row 64 = denom
  outT8     = fp8(pv * (2^11/denom broadcast))  (odd heads DMA-shifted to partitions 64:128)
  logitsT   = (outT8 [+ outT8r]).T @ lmW8 * 2^-22 + lm_b   (vocab on partitions)
"""

import sys

if "/opt/trn_rl_repo" not in sys.path:
    sys.path.insert(0, "/opt/trn_rl_repo")

import numpy as np

VOCAB = 32000
E = 1024
T = 1024
H = 16
HS = 64
B = 4
VSH = VOCAB // 2  # per-core vocab shard
NE = E // 128  # 8 e-tiles
NT = T // 128  # 8 t-tiles
NVT = VSH // 128  # 125 vocab tiles (128 wide, on partitions in phase 4)
FP8_SCALE = 2.0**11  # scale for both fp8 operands of the lm_head
DESCALE = float(2.0 ** (-22))

_cache = {}


def _build_nc(debug=False, tune=None):
    tune = dict(tune or {})

    def tn(k, d):
        return tune.get(k, d)

    import concourse.bass as bass
    import concourse.bacc as bacc
    import concourse.tile as tile
    from concourse import mybir
    from contextlib import ExitStack

    f32 = mybir.dt.float32
    f32r = mybir.dt.float32r
    f8 = mybir.dt.float8e4
    u8 = mybir.dt.uint8
    i32 = mybir.dt.int32

    nc = bacc.Bacc("TRN2", target_bir_lowering=False, debug=False)

    idx_d = nc.dram_tensor("idx", [128, NT], i32, kind="ExternalInput").ap()
    tok_d = nc.dram_tensor("tok", [VOCAB, E], f32, kind="ExternalInput").ap()
    post_d = nc.dram_tensor("post", [NE, 128, T], f32, kind="ExternalInput").ap()
    wq_d = nc.dram_tensor("wq", [8, NE, 128, 128], f32, kind="ExternalInput").ap()
    wk_d = nc.dram_tensor("wk", [8, NE, 128, 128], f32, kind="ExternalInput").ap()
    wv_d = nc.dram_tensor("wv", [NE, 128, E], f32, kind="ExternalInput").ap()
    msk_d = nc.dram_tensor("msk", [4, 128, 512], f32, kind="ExternalInput").ap()
    idn_d = nc.dram_tensor("idn", [128, 128], f32, kind="ExternalInput").ap()
    # fp8 lm_head weights, packed [vt, p, kp, s, m] and grouped by VG
    # vocab tiles per DMA: W8[kp*256 + s*128 + p, vt*128 + m]
    VG = tn("vg", 5)
    NVG = NVT // VG
    lmw8_d = nc.dram_tensor(
        "lmw8", [NVG, VG, 128, 4, 2, 128], u8, kind="ExternalInput"
    ).ap()
    # bias pre-transposed to [p, vt]
    lmbt_d = nc.dram_tensor("lmbt", [128, NVT], f32, kind="ExternalInput").ap()
    # logits transposed: [vocab_shard, tokens]; host transposes back
    out_d = nc.dram_tensor("logits", [NVG, VG, 128, T], f32, kind="ExternalOutput").ap()
    if debug:
        xt_o = nc.dram_tensor("xt_o", [128, NE, T], f32, kind="ExternalOutput").ap()
        va_o = nc.dram_tensor("va_o", [128, NT, H, HS + 1], f32, kind="ExternalOutput").ap()
        o8_o = nc.dram_tensor("o8_o", [128, NE, T], u8, kind="ExternalOutput").ap()
        o8r_o = nc.dram_tensor("o8r_o", [128, NE, 128], u8, kind="ExternalOutput").ap()

    EXP = mybir.ActivationFunctionType.Exp
    IDENT = mybir.ActivationFunctionType.Identity

    with tile.TileContext(nc) as tc, ExitStack() as ctx:
        const = ctx.enter_context(tc.tile_pool(name="const", bufs=1))
        persist = ctx.enter_context(tc.tile_pool(name="persist", bufs=1))

        idn = const.tile([128, 128], f32)
        nc.sync.dma_start(idn[:], idn_d[:])
        msk = const.tile([128, 4, 512], f32)
        nc.sync.dma_start(msk[:], msk_d.rearrange("k p n -> p k n"))
        idxt = const.tile([128, NT], i32)
        nc.sync.dma_start(idxt[:], idx_d[:])
        ones = const.tile([128, 1], f32)
        nc.vector.memset(ones[:], 1.0)
        # the out-normalization broadcast carries the fp8 up-scale for free
        sc64r = const.tile([128, 64], f32r)
        scv = const.tile([128, 1], f32)
        nc.vector.memset(scv[:], FP8_SCALE)
        nc.vector.tensor_copy(sc64r[:], scv[:].to_broadcast([128, 64]))

        xT = persist.tile([128, NE, T], f32r)
        outT8 = persist.tile([128, NE, T], f8)
        outT8r = persist.tile([128, NE, 128], f8)  # token-tile-0 fp8 residual
        outT0f = persist.tile([128, NE, 128], f32)  # token-tile-0 f32 scratch
        vaug = persist.tile([128, NT, H, HS + 1], f32r)

        # ---------- Phase 1: embedding gather + transpose + positional add
        with (
            tc.tile_pool(name="ph1", bufs=tn("ph1", 3)) as gp,
            tc.tile_pool(name="postp", bufs=1) as postp,
            tc.tile_pool(name="pst", bufs=tn("pst", 4), space="PSUM") as pst,
        ):
            post_sb = postp.tile([128, NE, T], f32)
            for k in range(NE):
                nc.sync.dma_start(post_sb[:, k, :], post_d[k])
            for tt in range(NT):
                xg = gp.tile([128, E], f32, tag="xg")
                nc.gpsimd.indirect_dma_start(
                    out=xg[:],
                    out_offset=None,
                    in_=tok_d[:],
                    in_offset=bass.IndirectOffsetOnAxis(
                        ap=idxt[:, tt : tt + 1], axis=0
                    ),
                )
                for k in range(NE):
                    ps = pst.tile([128, 128], f32, tag="tp")
                    nc.tensor.transpose(
                        out=ps[:], in_=xg[:, k * 128 : (k + 1) * 128], identity=idn[:]
                    )
                    nc.vector.tensor_add(
                        xT[:, k, tt * 128 : (tt + 1) * 128],
                        ps[:],
                        post_sb[:, k, tt * 128 : (tt + 1) * 128],
                    )

        # ---------- Phase 2: V for all heads (natural layout) + vaug build
        with (
            tc.tile_pool(name="wvp", bufs=1) as wvp,
            tc.tile_pool(name="pv2", bufs=tn("pv2", 4), space="PSUM") as pv2,
        ):
            wv_sb = wvp.tile([128, NE, E], f32r)
            for k in range(NE):
                nc.sync.dma_start(wv_sb[:, k, :], wv_d[k].bitcast(f32r))
            for st in range(NT):
                for ns in range(2):
                    ps = pv2.tile([128, 512], f32, tag="vps")
                    for k in range(NE):
                        nc.tensor.matmul(
                            ps[:],
                            xT[:, k, st * 128 : (st + 1) * 128],
                            wv_sb[:, k, ns * 512 : (ns + 1) * 512],
                            start=(k == 0),
                            stop=(k == NE - 1),
                        )
                    for hh in range(8):
                        h = ns * 8 + hh
                        nc.vector.tensor_copy(
                            vaug[:, st, h, 0:HS], ps[:, hh * 64 : (hh + 1) * 64]
                        )
                nc.vector.tensor_copy(
                    vaug[:, st, :, HS : HS + 1], ones[:].to_broadcast([128, H, 1])
                )

        if debug:
            nc.sync.dma_start(xt_o[:], xT[:].bitcast(f32))
            nc.sync.dma_start(va_o[:], vaug[:].bitcast(f32))

        # ---------- Phase 3: attention, one head pair at a time
        with (
            tc.tile_pool(name="wqk", bufs=tn("wqk", 3)) as wqk,
            tc.tile_pool(name="qkp", bufs=tn("qkp", 2)) as qkp,
            tc.tile_pool(name="ep", bufs=tn("ep", 9)) as ep,
            tc.tile_pool(name="rp", bufs=tn("rp", 4)) as rp,
            tc.tile_pool(name="sp", bufs=tn("sp", 4)) as sp,
            tc.tile_pool(name="pqk", bufs=tn("pqk", 2), space="PSUM") as pqk,
            tc.tile_pool(name="psc", bufs=tn("psc", 3), space="PSUM") as psc,
            tc.tile_pool(name="ppv", bufs=tn("ppv", 2), space="PSUM") as ppv,
            tc.tile_pool(name="pbc", bufs=tn("pbc", 1), space="PSUM") as pbc,
        ):
            for pr in range(8):
                qT = qkp.tile([128, T], f32r, tag="qT")
                kT = qkp.tile([128, T], f32r, tag="kT")
                for wd, dst, wtag in ((wq_d, qT, "wq"), (wk_d, kT, "wk")):
                    w_sb = wqk.tile([128, NE, 128], f32r, tag=wtag)
                    nc.sync.dma_start(w_sb[:], wd[pr].rearrange("k p n -> p k n").bitcast(f32r))
                    for ts2 in range(2):
                        ps = pqk.tile([128, 512], f32, tag="qkps")
                        for k in range(NE):
                            nc.tensor.matmul(
                                ps[:],
                                w_sb[:, k, :],
                                xT[:, k, ts2 * 512 : (ts2 + 1) * 512],
                                start=(k == 0),
                                stop=(k == NE - 1),
                            )
                        if tn("qk_act", 1):
                            nc.scalar.activation(
                                dst[:, ts2 * 512 : (ts2 + 1) * 512], ps[:], IDENT
                            )
                        else:
                            nc.vector.tensor_copy(dst[:, ts2 * 512 : (ts2 + 1) * 512], ps[:])
                for sub in range(2):
                    h = 2 * pr + sub
                    q_s = qT[sub * 64 : (sub + 1) * 64, :]
                    k_s = kT[sub * 64 : (sub + 1) * 64, :]
                    for j in range(2):
                        smax = 4 * j + 3
                        e_tiles = []
                        for i in range(smax + 1):
                            ko = i - 4 * j
                            # causal trim: columns below the diagonal block
                            # are fully masked; skip them on PE/ACT/DVE
                            off = 128 * ko if (ko > 0 and tn("trim", 1)) else 0
                            ps = psc.tile([128, 512], f32, tag="sc")
                            nc.tensor.matmul(
                                ps[:, off:512],
                                k_s[:, i * 128 : (i + 1) * 128],
                                q_s[:, j * 512 + off : (j + 1) * 512],
                                start=True,
                                stop=True,
                            )
                            et = ep.tile([128, 512], f32r, tag="e")
                            nc.scalar.activation(et[:, off:512], ps[:, off:512], EXP)
                            if ko >= 0:
                                # alternate mask engine DVE/Pool to balance
                                meng = nc.gpsimd if (i % 2 == tn("mpar", 1)) else nc.vector
                                meng.tensor_mul(
                                    et[:, off:512], et[:, off:512], msk[:, ko, off:512]
                                )
                            e_tiles.append((off, et))
                        pv = ppv.tile([128, 512], f32, tag="pv")
                        for n, (off, et) in enumerate(e_tiles):
                            nc.tensor.matmul(
                                pv[0:65, off:512],
                                vaug[:, n, h, :],
                                et[:, off:512],
                                start=(n == 0),
                                stop=(n == smax),
                                skip_group_check=True,
                            )
                        rc = rp.tile([128, 512], f32, tag="rc")
                        # custom-DVE reciprocal can't read PSUM on HW (CoreSim
                        # diverges); stage the denominator row into SBUF first
                        nc.vector.reciprocal(rc[64:65, :], pv[64:65, :])
                        # partition-broadcast row 64 -> rows 0:64 via a K=1
                        # matmul (sc64[64] outer recip_row); DMA-based SBUF
                        # broadcasts race on HW. sc64r = 2^11 folds in the
                        # fp8 up-scale.
                        rcr = rp.tile([128, 512], f32r, tag="rcr")
                        nc.vector.tensor_copy(rcr[64:65, :], rc[64:65, :])
                        pb = pbc.tile([128, 512], f32, tag="pb")
                        nc.tensor.matmul(
                            pb[0:64, :],
                            sc64r[64:65, :],
                            rcr[64:65, :],
                            start=True,
                            stop=True,
                        )
                        rcb = rp.tile([128, 512], f32, tag="rcb")
                        (nc.gpsimd if tn("rcb_pool", 0) else nc.vector).tensor_copy(
                            rcb[0:64, :], pb[0:64, :]
                        )
                        jsl = slice(j * 512, (j + 1) * 512)
                        if sub == 0:
                            nc.vector.tensor_mul(
                                outT8[0:64, pr, jsl], pv[0:64, :], rcb[0:64, :]
                            )
                            if j == 0:
                                nc.vector.tensor_mul(
                                    outT0f[0:64, pr, :], pv[0:64, 0:128], rcb[0:64, 0:128]
                                )
                                rsb = sp.tile([128, 128], f32, tag="rsb")
                                nc.vector.tensor_sub(
                                    rsb[0:64, :], outT0f[0:64, pr, :], outT8[0:64, pr, 0:128]
                                )
                                nc.vector.tensor_copy(outT8r[0:64, pr, :], rsb[0:64, :])
                        else:
                            stg = sp.tile([128, 512], f8, tag="stg")
                            nc.vector.tensor_mul(stg[0:64, :], pv[0:64, :], rcb[0:64, :])
                            nc.sync.dma_start(outT8[64:128, pr, jsl], stg[0:64, :])
                            if j == 0:
                                nc.vector.tensor_mul(
                                    outT0f[0:64, pr, :], pv[0:64, 0:128], rcb[0:64, 0:128]
                                )
                                rsb = sp.tile([128, 128], f32, tag="rsb")
                                nc.vector.tensor_sub(
                                    rsb[0:64, :], outT0f[0:64, pr, :], stg[0:64, 0:128]
                                )
                                rs8 = sp.tile([128, 128], f8, tag="rs8")
                                nc.vector.tensor_copy(rs8[0:64, :], rsb[0:64, :])
                                nc.sync.dma_start(outT8r[64:128, pr, :], rs8[0:64, :])

        if debug:
            nc.sync.dma_start(o8_o[:], outT8[:].bitcast(u8))
            nc.sync.dma_start(o8r_o[:], outT8r[:].bitcast(u8))

        # ---------- Phase 4: lm head, fp8 DoubleRow, [vocab, token] layout.
        # DMAs are grouped over VG vocab tiles: HWDGE charges a fixed
        # ~625ns per DMA instruction, so fewer/bigger transfers matter.
        with (
            tc.tile_pool(name="lwp", bufs=tn("lwp", 3)) as lwp,
            tc.tile_pool(name="bp", bufs=1) as bp,
            tc.tile_pool(name="ogp", bufs=tn("ogp", 2)) as ogp,
            tc.tile_pool(name="plm", bufs=tn("plm", 6), space="PSUM") as plm,
        ):
            lmb_sb = bp.tile([128, NVT], f32)
            nc.sync.dma_start(lmb_sb[:], lmbt_d[:])
            for vg in range(NVG):
                lw = lwp.tile([128, VG, 4, 2, 128], f8, tag="lw")
                nc.sync.dma_start(
                    lw[:], lmw8_d[vg].rearrange("g p a b c -> p g a b c").bitcast(f8)
                )
                og = ogp.tile([128, VG, T], f32, tag="og")
                for g in range(VG):
                    vt = vg * VG + g
                    for tt in range(2):
                        ps = plm.tile([128, 512], f32, tag="lm")
                        tsl = slice(tt * 512, (tt + 1) * 512)
                        for kp in range(4):
                            nc.tensor.matmul(
                                ps[:],
                                lw[:, g, kp],
                                outT8[:, 2 * kp : 2 * kp + 2, tsl],
                                start=(kp == 0),
                                stop=(kp == 3 and tt == 1),
                                perf_mode=mybir.MatmulPerfMode.DoubleRow,
                            )
                        if tt == 0:
                            # token-tile-0 residual correction, same PSUM group
                            for kp in range(4):
                                nc.tensor.matmul(
                                    ps[:, 0:128],
                                    lw[:, g, kp],
                                    outT8r[:, 2 * kp : 2 * kp + 2, :],
                                    start=False,
                                    stop=(kp == 3),
                                    perf_mode=mybir.MatmulPerfMode.DoubleRow,
                                    skip_group_check=True,
                                )
                        nc.scalar.activation(
                            og[:, g, tsl],
                            ps[:],
                            IDENT,
                            bias=lmb_sb[:, vt : vt + 1],
                            scale=DESCALE,
                        )
                nc.sync.dma_start(
                    out_d[vg].rearrange("g p t -> p g t"), og[:]
                )

    nc.compile()
    return nc


def _prep_shared(tok_emb, pos_emb, Wq, Wk, Wv):
    f = np.float32
    tok = np.ascontiguousarray(tok_emb, dtype=f)
    post = np.ascontiguousarray(pos_emb.T.reshape(NE, 128, T), dtype=f)

    def pair_stack(W):
        out = np.empty((8, NE, 128, 128), dtype=f)
        for p in range(8):
            pairw = np.concatenate([W[2 * p], W[2 * p + 1]], axis=1)  # [E, 128]
            out[p] = pairw.reshape(NE, 128, 128)
        return out

    wq = pair_stack(np.asarray(Wq, dtype=f))
    wk = pair_stack(np.asarray(Wk, dtype=f))
    wv = np.ascontiguousarray(
        np.asarray(Wv, dtype=f).transpose(1, 0, 2).reshape(E, H * HS).reshape(NE, 128, E)
    )
    # causal masks: msk[ko][p, c] = 1.0 if p + 128*ko <= c else 0.0
    p_idx = np.arange(128)[:, None]
    c_idx = np.arange(512)[None, :]
    msk = np.stack(
        [(p_idx + 128 * ko <= c_idx).astype(f) for ko in range(4)]
    )
    idn = np.eye(128, dtype=f)
    return {"tok": tok, "post": post, "wq": wq, "wk": wk, "wv": wv, "msk": msk, "idn": idn}


def _prep_lm(lm_W, lm_b):
    """Quantize + pack the lm_head for the fp8 DoubleRow phase 4.

    Returns per-vocab-half (lmw8 [NVT,128,4,2,128] u8, lmbt [128,NVT] f32)."""
    import ml_dtypes

    outs = []
    for v in range(2):
        Wsh = np.asarray(lm_W[:, v * VSH : (v + 1) * VSH], dtype=np.float32)
        W8 = np.clip(Wsh * FP8_SCALE, -240.0, 240.0).astype(ml_dtypes.float8_e4m3)
        # [E, VSH] -> [kp, s, p, vt, m] -> [vt, p, kp, s, m]
        t = W8.reshape(4, 2, 128, NVT, 128)
        lmw8 = (
            np.ascontiguousarray(t.transpose(3, 2, 0, 1, 4))
            .view(np.uint8)
            .reshape(NVT // 5, 5, 128, 4, 2, 128)
        )
        bsh = np.asarray(lm_b[v * VSH : (v + 1) * VSH], dtype=np.float32)
        lmbt = np.ascontiguousarray(bsh.reshape(NVT, 128).T)
        outs.append((lmw8, lmbt))
    return outs


def kernel(idx, tok_emb, pos_emb, Wq, Wk, Wv, lm_W, lm_b):
    from concourse.bass_utils import run_bass_kernel_spmd

    if "nc" not in _cache:
        _cache["nc"] = _build_nc()
    nc = _cache["nc"]

    idx = np.asarray(idx)
    shared = _prep_shared(
        np.asarray(tok_emb), np.asarray(pos_emb), np.asarray(Wq), np.asarray(Wk), np.asarray(Wv)
    )
    lm_parts = _prep_lm(np.asarray(lm_W), np.asarray(lm_b))

    in_maps = []
    for c in range(8):
        b, v = c // 2, c % 2
        m = dict(shared)
        m["idx"] = np.ascontiguousarray(
            idx[b].astype(np.int32).reshape(NT, 128).T
        )
        m["lmw8"], m["lmbt"] = lm_parts[v]
        in_maps.append(m)

    res = run_bass_kernel_spmd(nc, in_maps, core_ids=list(range(8)))
    logits = np.empty((B, T, VOCAB), np.float32)
    for c in range(8):
        b, v = c // 2, c % 2
        lg = res.results[c]["logits"].reshape(VSH, T)
        logits[b, :, v * VSH : (v + 1) * VSH] = lg.T
    return logits


# revision 15
# speedup vs baseline: 1.2496x; 1.2496x over previous
"""Trainium2 Bass kernel for a single-layer bigram language model
(embed + 16-head causal attention + vocab lm_head).

Sharding: 8 cores = 4 batches x 2 vocab halves. Core c handles batch
c//2, vocab columns [(c%2)*16000, ...), and computes attention for 8 of
the 16 heads (even cores: heads 0-7, odd: 8-15 -- pure SPMD, the head
assignment comes from which Wq/Wk/Wv slices each core receives). The
fp8 attention output (+ its token-tile-0 residual) is exchanged between
the two cores of a batch pair with two small AllGathers (pair replica
groups), pipelined against the second half of the attention loop.

The lm_head runs in fp8 (e4m3) DoubleRow perf mode (two contraction
rows per PE column-cycle => 2x bf16 throughput). Both operands are
scaled by 2^11 before quantization; the product descale 2^-22 plus the
bias add are folded into one Activation op per output tile. Accuracy:
the max-abs error of full-fp8 logits is driven entirely by the first
~128 tokens (softmax over few positions => large attention-out rows).
Fix: an fp8 *residual* of the attention out for token tile 0 (same 2^11
scale, so it accumulates into the same PSUM group) adds a second-order
correction only where needed. Measured on HW: rel err 1.63e-2 (gate
2e-2); full-fp8 without the correction would be 2.2e-2.

Phase-4 layout is [vocab_partition, token_free]:
  - bias becomes per-partition => fused into the ACT descale op
  - logits DMA out as [VSH, T]; the host transposes when assembling.
Phase-4 DMAs are grouped over VG=5 vocab tiles (HWDGE charges a fixed
~625ns per DMA instruction) and the entire 16MB fp8 lm_W prefetches
during the attention phase (lwp bufs=25 covers all 25 groups).

All other matmuls run as float32r (fp32 data, PE truncates to fp22)
which runs at full bf16 speed for moving dims >= 256, with fp32 PSUM
accumulation. Attention epilogue work is spread across DVE (normalize,
residual), ACT (exp, q/k PSUM->SBUF copies), and Pool/GPSIMD (half the
causal-mask multiplies) to keep DVE off the critical path.

Layout strategy: everything downstream of the embedding is kept
transposed ([feature, token]) so all matmul contractions have their K
dim on partitions:
  xT[e, t]  = (tok_emb[idx] + pos_emb).T        (PE-transposed after gather)
  qT/kT     = Wq_pair.T @ xT  (pairs of heads stacked on partitions 0:64 / 64:128)
  scoresT   = kT_h.T-slices @ qT_h  -> [s, t] tiles, causally trimmed;
              exp on ACT; boundary masks on DVE/Pool
  pv        = [v_h | 1].T @ e       -> rows 0:64 = unnormalized outT, row 64 = denom
  outT8     = fp8(pv * (2^11/denom broadcast))  (odd heads DMA-shifted to partitions 64:128)
  logitsT   = (outT8 [+ outT8r]).T @ lmW8 * 2^-22 + lm_b   (vocab on partitions)

TimelineSim estimate: ~390us/core (baseline f32r version: ~718us).
"""

import sys

if "/opt/trn_rl_repo" not in sys.path:
    sys.path.insert(0, "/opt/trn_rl_repo")

import numpy as np

VOCAB = 32000
E = 1024
T = 1024
H = 16
HS = 64
B = 4
VSH = VOCAB // 2  # per-core vocab shard
NE = E // 128  # 8 e-tiles
NT = T // 128  # 8 t-tiles
NVT = VSH // 128  # 125 vocab tiles (128 wide, on partitions in phase 4)
FP8_SCALE = 2.0**11  # scale for both fp8 operands of the lm_head
DESCALE = float(2.0 ** (-22))

_cache = {}


def _build_nc(tune=None):
    tune = dict(tune or {})

    def tn(k, d):
        return tune.get(k, d)

    import concourse.bass as bass
    import concourse.bacc as bacc
    import concourse.tile as tile
    from concourse import mybir
    from contextlib import ExitStack

    f32 = mybir.dt.float32
    f32r = mybir.dt.float32r
    f8 = mybir.dt.float8e4
    u8 = mybir.dt.uint8
    i32 = mybir.dt.int32

    nc = bacc.Bacc("TRN2", target_bir_lowering=False, debug=False)

    idx_d = nc.dram_tensor("idx", [128, NT], i32, kind="ExternalInput").ap()
    tok_d = nc.dram_tensor("tok", [VOCAB, E], f32, kind="ExternalInput").ap()
    post_d = nc.dram_tensor("post", [NE, 128, T], f32, kind="ExternalInput").ap()
    HSPL = tn("hsplit", 1)
    NPR = 4 if HSPL else 8  # head-pairs computed locally
    NHL = 2 * NPR  # local heads
    wq_d = nc.dram_tensor("wq", [NPR, NE, 128, 128], f32, kind="ExternalInput").ap()
    wk_d = nc.dram_tensor("wk", [NPR, NE, 128, 128], f32, kind="ExternalInput").ap()
    wv_d = nc.dram_tensor("wv", [NE, 128, 64 * NHL], f32, kind="ExternalInput").ap()
    msk_d = nc.dram_tensor("msk", [4, 128, 512], f32, kind="ExternalInput").ap()
    idn_d = nc.dram_tensor("idn", [128, 128], f32, kind="ExternalInput").ap()
    # fp8 lm_head weights, packed [vt, p, kp, s, m] and grouped by VG
    # vocab tiles per DMA: W8[kp*256 + s*128 + p, vt*128 + m]
    VG = tn("vg", 5)
    NVG = NVT // VG
    lmw8_d = nc.dram_tensor(
        "lmw8", [NVG, VG, 128, 4, 2, 128], u8, kind="ExternalInput"
    ).ap()
    # bias pre-transposed to [p, vt]
    lmbt_d = nc.dram_tensor("lmbt", [128, NVT], f32, kind="ExternalInput").ap()
    # logits transposed: [vocab_shard, tokens]; host transposes back
    out_d = nc.dram_tensor("logits", [NVG, VG, 128, T], f32, kind="ExternalOutput").ap()

    EXP = mybir.ActivationFunctionType.Exp
    IDENT = mybir.ActivationFunctionType.Identity

    with tile.TileContext(nc) as tc, ExitStack() as ctx:
        const = ctx.enter_context(tc.tile_pool(name="const", bufs=1))
        persist = ctx.enter_context(tc.tile_pool(name="persist", bufs=1))

        idxt = const.tile([128, NT], i32)
        nc.sync.dma_start(idxt[:], idx_d[:])
        idn = const.tile([128, 128], f32)
        nc.sync.dma_start(idn[:], idn_d[:])
        msk = const.tile([128, 4, 512], f32)
        ones = const.tile([128, 1], f32)
        nc.vector.memset(ones[:], 1.0)
        # the out-normalization broadcast carries the fp8 up-scale for free
        sc64r = const.tile([128, 64], f32r)
        scv = const.tile([128, 1], f32)
        nc.vector.memset(scv[:], FP8_SCALE)
        nc.vector.tensor_copy(sc64r[:], scv[:].to_broadcast([128, 64]))

        outT8 = persist.tile([128, NE, T], f8)
        outT8r = persist.tile([128, NE, 128], f8)  # token-tile-0 fp8 residual
        # attention-lifetime tensors live in their own pool so the SBUF is
        # reclaimed before phase 4 (frees room for the full lm_W8 prefetch)
        attnp_cm = tc.tile_pool(name="attn", bufs=1)
        attnp = attnp_cm.__enter__()
        xT = attnp.tile([128, NE, T], f32r)
        outT0f = attnp.tile([128, NPR, 128], f32)  # token-tile-0 f32 scratch
        vaug = attnp.tile([128, NT, NHL, HS + 1], f32r)

        # ---------- Phase 1+2 interleaved: per token tile, gather ->
        # transpose+positional add -> V matmul. Interleaving keeps the
        # (in-order) PE busy with V work during the gather-wait gaps.
        with (
            tc.tile_pool(name="ph1", bufs=tn("ph1", 8)) as gp,
            tc.tile_pool(name="postp", bufs=1) as postp,
            tc.tile_pool(name="wvp", bufs=1) as wvp,
            tc.tile_pool(name="pst", bufs=tn("pst", 4), space="PSUM") as pst,
            tc.tile_pool(name="pv2", bufs=tn("pv2", 4), space="PSUM") as pv2,
        ):
            post_sb = postp.tile([128, NE, T], f32)
            wv_sb = wvp.tile([128, NE, 64 * NHL], f32r)
            xgs = {}

            def issue_gather(tt):
                xg = gp.tile([128, E], f32, tag="xg")
                nc.gpsimd.indirect_dma_start(
                    out=xg[:],
                    out_offset=None,
                    in_=tok_d[:],
                    in_offset=bass.IndirectOffsetOnAxis(
                        ap=idxt[:, tt : tt + 1], axis=0
                    ),
                )
                return xg

            for tt in range(NT if tn("pregather", 1) else 1):
                xgs[tt] = issue_gather(tt)
            for k in range(NE):
                (nc.scalar if tn("post_act", 0) else nc.sync).dma_start(
                    post_sb[:, k, :], post_d[k]
                )
            for k in range(NE):
                (nc.scalar if tn("wv_act", 0) else nc.sync).dma_start(
                    wv_sb[:, k, :], wv_d[k].bitcast(f32r)
                )
            for tt in range(NT):
                xg = xgs.get(tt) or issue_gather(tt)
                for k in range(NE):
                    ps = pst.tile([128, 128], f32, tag="tp")
                    nc.tensor.transpose(
                        out=ps[:], in_=xg[:, k * 128 : (k + 1) * 128], identity=idn[:]
                    )
                    nc.vector.tensor_add(
                        xT[:, k, tt * 128 : (tt + 1) * 128],
                        ps[:],
                        post_sb[:, k, tt * 128 : (tt + 1) * 128],
                    )
                st = tt
                for ns in range(NHL // 8):
                    ps = pv2.tile([128, 512], f32, tag="vps")
                    for k in range(NE):
                        nc.tensor.matmul(
                            ps[:],
                            xT[:, k, st * 128 : (st + 1) * 128],
                            wv_sb[:, k, ns * 512 : (ns + 1) * 512],
                            start=(k == 0),
                            stop=(k == NE - 1),
                        )
                    for hh in range(8):
                        h = ns * 8 + hh
                        if tn("vaug_act", 1):
                            nc.scalar.activation(
                                vaug[:, st, h, 0:HS], ps[:, hh * 64 : (hh + 1) * 64], IDENT
                            )
                        else:
                            nc.vector.tensor_copy(
                                vaug[:, st, h, 0:HS], ps[:, hh * 64 : (hh + 1) * 64]
                            )
                nc.vector.tensor_copy(
                    vaug[:, st, :, HS : HS + 1], ones[:].to_broadcast([128, NHL, 1])
                )

        # ---------- Phase 3: attention, one head pair at a time
        nc.sync.dma_start(msk[:], msk_d.rearrange("k p n -> p k n"))
        with (
            tc.tile_pool(name="wqk", bufs=tn("wqk", 3)) as wqk,
            tc.tile_pool(name="qkp", bufs=tn("qkp", 2)) as qkp,
            tc.tile_pool(name="ep", bufs=tn("ep", 9)) as ep,
            tc.tile_pool(name="rp", bufs=tn("rp", 4)) as rp,
            tc.tile_pool(name="sp", bufs=tn("sp", 4)) as sp,
            tc.tile_pool(name="pqk", bufs=tn("pqk", 2), space="PSUM") as pqk,
            tc.tile_pool(name="psc", bufs=tn("psc", 3), space="PSUM") as psc,
            tc.tile_pool(name="ppv", bufs=tn("ppv", 2), space="PSUM") as ppv,
            tc.tile_pool(name="pbc", bufs=tn("pbc", 1), space="PSUM") as pbc,
        ):
            ccp = ctx.enter_context(
                tc.tile_pool(name="ccp", bufs=1, space="DRAM")
            )
            PW = T + 128  # packed width: outT8 row + residual row
            for pr in range(NPR):
                qT = qkp.tile([128, T], f32r, tag="qT")
                kT = qkp.tile([128, T], f32r, tag="kT")
                for wd, dst, wtag in ((wq_d, qT, "wq"), (wk_d, kT, "wk")):
                    w_sb = wqk.tile([128, NE, 128], f32r, tag=wtag)
                    nc.sync.dma_start(w_sb[:], wd[pr].rearrange("k p n -> p k n").bitcast(f32r))
                    for ts2 in range(2):
                        ps = pqk.tile([128, 512], f32, tag="qkps")
                        for k in range(NE):
                            nc.tensor.matmul(
                                ps[:],
                                w_sb[:, k, :],
                                xT[:, k, ts2 * 512 : (ts2 + 1) * 512],
                                start=(k == 0),
                                stop=(k == NE - 1),
                            )
                        if tn("qk_act", 1):
                            nc.scalar.activation(
                                dst[:, ts2 * 512 : (ts2 + 1) * 512], ps[:], IDENT
                            )
                        else:
                            nc.vector.tensor_copy(dst[:, ts2 * 512 : (ts2 + 1) * 512], ps[:])
                for sub in range(2):
                    h = 2 * pr + sub
                    q_s = qT[sub * 64 : (sub + 1) * 64, :]
                    k_s = kT[sub * 64 : (sub + 1) * 64, :]
                    for j in range(2):
                        smax = 4 * j + 3
                        e_tiles = []
                        for i in range(smax + 1):
                            ko = i - 4 * j
                            # causal trim: columns below the diagonal block
                            # are fully masked; skip them on PE/ACT/DVE
                            off = 128 * ko if (ko > 0 and tn("trim", 1)) else 0
                            ps = psc.tile([128, 512], f32, tag="sc")
                            nc.tensor.matmul(
                                ps[:, off:512],
                                k_s[:, i * 128 : (i + 1) * 128],
                                q_s[:, j * 512 + off : (j + 1) * 512],
                                start=True,
                                stop=True,
                            )
                            et = ep.tile([128, 512], f32r, tag="e")
                            nc.scalar.activation(et[:, off:512], ps[:, off:512], EXP)
                            if ko >= 0:
                                # alternate mask engine DVE/Pool to balance
                                meng = nc.gpsimd if (i % 2 == tn("mpar", 1)) else nc.vector
                                meng.tensor_mul(
                                    et[:, off:512], et[:, off:512], msk[:, ko, off:512]
                                )
                            e_tiles.append((off, et))
                        pv = ppv.tile([128, 512], f32, tag="pv")
                        for n, (off, et) in enumerate(e_tiles):
                            nc.tensor.matmul(
                                pv[0:65, off:512],
                                vaug[:, n, h, :],
                                et[:, off:512],
                                start=(n == 0),
                                stop=(n == smax),
                                skip_group_check=True,
                            )
                        rc = rp.tile([128, 512], f32, tag="rc")
                        # custom-DVE reciprocal can't read PSUM on HW (CoreSim
                        # diverges); stage the denominator row into SBUF first
                        nc.vector.reciprocal(rc[64:65, :], pv[64:65, :])
                        # partition-broadcast row 64 -> rows 0:64 via a K=1
                        # matmul (sc64[64] outer recip_row); DMA-based SBUF
                        # broadcasts race on HW. sc64r = 2^11 folds in the
                        # fp8 up-scale.
                        rcr = rp.tile([128, 512], f32r, tag="rcr")
                        nc.vector.tensor_copy(rcr[64:65, :], rc[64:65, :])
                        pb = pbc.tile([128, 512], f32, tag="pb")
                        nc.tensor.matmul(
                            pb[0:64, :],
                            sc64r[64:65, :],
                            rcr[64:65, :],
                            start=True,
                            stop=True,
                        )
                        rcb = rp.tile([128, 512], f32, tag="rcb")
                        (nc.gpsimd if tn("rcb_pool", 0) else nc.vector).tensor_copy(
                            rcb[0:64, :], pb[0:64, :]
                        )
                        jsl = slice(j * 512, (j + 1) * 512)
                        if sub == 0:
                            nc.vector.tensor_mul(
                                outT8[0:64, pr, jsl], pv[0:64, :], rcb[0:64, :]
                            )
                            if j == 0:
                                nc.vector.tensor_mul(
                                    outT0f[0:64, pr, :], pv[0:64, 0:128], rcb[0:64, 0:128]
                                )
                                rsb = sp.tile([128, 128], f32, tag="rsb")
                                nc.vector.tensor_sub(
                                    rsb[0:64, :], outT0f[0:64, pr, :], outT8[0:64, pr, 0:128]
                                )
                                nc.vector.tensor_copy(outT8r[0:64, pr, :], rsb[0:64, :])
                        else:
                            stg = sp.tile([128, 512], f8, tag="stg")
                            nc.vector.tensor_mul(stg[0:64, :], pv[0:64, :], rcb[0:64, :])
                            nc.sync.dma_start(outT8[64:128, pr, jsl], stg[0:64, :])
                            if j == 0:
                                nc.vector.tensor_mul(
                                    outT0f[0:64, pr, :], pv[0:64, 0:128], rcb[0:64, 0:128]
                                )
                                rsb = sp.tile([128, 128], f32, tag="rsb")
                                nc.vector.tensor_sub(
                                    rsb[0:64, :], outT0f[0:64, pr, :], stg[0:64, 0:128]
                                )
                                rs8 = sp.tile([128, 128], f8, tag="rs8")
                                nc.vector.tensor_copy(rs8[0:64, :], rsb[0:64, :])
                                nc.sync.dma_start(outT8r[64:128, pr, :], rs8[0:64, :])
                CCN = tn("ccn", 2)  # head-pairs per exchange collective
                if CCN == 3:
                    # asymmetric: bulk exchange after pr2, small tail after pr3
                    fire = {2: (0, 0, 3), 3: (1, 3, 4)}.get(pr)
                else:
                    fire = (
                        (pr // CCN, CCN * (pr // CCN), CCN * (pr // CCN) + CCN)
                        if pr % CCN == CCN - 1
                        else None
                    )
                if HSPL and fire is not None:
                    # local pairs done: AllGather with the partner core.
                    # Pack outT8 + residual into one buffer.
                    half, klo, khi = fire
                    CCW = khi - klo
                    ksl = slice(klo, khi)
                    cin = ccp.tile([128, CCW, PW], f8, space="DRAM", tag=f"cin{half}")
                    cout = ccp.tile(
                        [2, 128, CCW, PW], f8, space="DRAM", tag=f"cout{half}"
                    )
                    nc.sync.dma_start(cin[:, :, 0:T], outT8[:, ksl, :])
                    nc.sync.dma_start(cin[:, :, T:PW], outT8r[:, ksl, :])
                    nc.gpsimd.collective_compute(
                        kind="AllGather",
                        op=mybir.AluOpType.bypass,
                        replica_groups=[[0, 1], [2, 3], [4, 5], [6, 7]],
                        ins=[cin[:].bitcast(u8)],
                        outs=[cout[:].bitcast(u8)],
                    )
                    # unpack both ranks: SPMD means local heads may sit in
                    # the other global half (odd cores own heads 8..15)
                    for r in range(2):
                        gk = slice(4 * r + klo, 4 * r + khi)
                        nc.sync.dma_start(outT8[:, gk, :], cout[r, :, :, 0:T])
                        nc.sync.dma_start(outT8r[:, gk, :], cout[r, :, :, T:PW])

        attnp_cm.__exit__(None, None, None)

        # ---------- Phase 4: lm head, fp8 DoubleRow, [vocab, token] layout.
        # DMAs are grouped over VG vocab tiles: HWDGE charges a fixed
        # ~625ns per DMA instruction, so fewer/bigger transfers matter.
        with (
            tc.tile_pool(name="lwp", bufs=tn("lwp", 25)) as lwp,
            tc.tile_pool(name="bp", bufs=1) as bp,
            tc.tile_pool(name="ogp", bufs=tn("ogp", 3)) as ogp,
            tc.tile_pool(name="plm", bufs=tn("plm", 8), space="PSUM") as plm,
        ):
            lmb_sb = bp.tile([128, NVT], f32)
            nc.sync.dma_start(lmb_sb[:], lmbt_d[:])
            for vg in range(NVG):
                lw = lwp.tile([128, VG, 4, 2, 128], f8, tag="lw")
                # issue lm_W prefetch on the ACT DGE queue so it is
                # not FIFO-ordered behind the og output stores on SP's queue
                (nc.scalar if tn("lw_act", 0) else nc.sync).dma_start(
                    lw[:], lmw8_d[vg].rearrange("g p a b c -> p g a b c").bitcast(f8)
                )
                og = ogp.tile([128, VG, T], f32, tag="og")
                for g in range(VG):
                    vt = vg * VG + g
                    for tt in range(2):
                        ps = plm.tile([128, 512], f32, tag="lm")
                        tsl = slice(tt * 512, (tt + 1) * 512)
                        for kp in range(4):
                            nc.tensor.matmul(
                                ps[:],
                                lw[:, g, kp],
                                outT8[:, 2 * kp : 2 * kp + 2, tsl],
                                start=(kp == 0),
                                stop=(kp == 3 and tt == 1),
                                perf_mode=mybir.MatmulPerfMode.DoubleRow,
                            )
                        if tt == 0:
                            # token-tile-0 residual correction, same PSUM group
                            for kp in range(4):
                                nc.tensor.matmul(
                                    ps[:, 0:128],
                                    lw[:, g, kp],
                                    outT8r[:, 2 * kp : 2 * kp + 2, :],
                                    start=False,
                                    stop=(kp == 3),
                                    perf_mode=mybir.MatmulPerfMode.DoubleRow,
                                    skip_group_check=True,
                                )
                        if tn("og_dve", 0) and (2 * g + tt) % 2 == 1:
                            nc.vector.scalar_tensor_tensor(
                                og[:, g, tsl],
                                ps[:],
                                DESCALE,
                                lmb_sb[:, vt : vt + 1].to_broadcast([128, 512]),
                                mybir.AluOpType.mult,
                                mybir.AluOpType.add,
                            )
                        else:
                            nc.scalar.activation(
                                og[:, g, tsl],
                                ps[:],
                                IDENT,
                                bias=lmb_sb[:, vt : vt + 1],
                                scale=DESCALE,
                            )
                if tn("og_split", 0):
                    for g in range(VG):
                        nc.sync.dma_start(out_d[vg, g], og[:, g, :])
                else:
                    nc.sync.dma_start(
                        out_d[vg].rearrange("g p t -> p g t"), og[:]
                    )

    nc.compile()
    return nc


def _prep_shared(tok_emb, pos_emb, Wq, Wk, Wv):
    f = np.float32
    tok = np.ascontiguousarray(tok_emb, dtype=f)
    post = np.ascontiguousarray(pos_emb.T.reshape(NE, 128, T), dtype=f)

    def pair_stack(W):
        out = np.empty((8, NE, 128, 128), dtype=f)
        for p in range(8):
            pairw = np.concatenate([W[2 * p], W[2 * p + 1]], axis=1)  # [E, 128]
            out[p] = pairw.reshape(NE, 128, 128)
        return out

    wq_full = pair_stack(np.asarray(Wq, dtype=f))
    wk_full = pair_stack(np.asarray(Wk, dtype=f))
    wv_full = np.ascontiguousarray(
        np.asarray(Wv, dtype=f).transpose(1, 0, 2).reshape(E, H * HS).reshape(NE, 128, E)
    )
    # causal masks: msk[ko][p, c] = 1.0 if p + 128*ko <= c else 0.0
    p_idx = np.arange(128)[:, None]
    c_idx = np.arange(512)[None, :]
    msk = np.stack(
        [(p_idx + 128 * ko <= c_idx).astype(f) for ko in range(4)]
    )
    idn = np.eye(128, dtype=f)
    shared = {"tok": tok, "post": post, "msk": msk, "idn": idn}
    halves = []
    for hf in range(2):
        halves.append({
            "wq": np.ascontiguousarray(wq_full[4 * hf : 4 * hf + 4]),
            "wk": np.ascontiguousarray(wk_full[4 * hf : 4 * hf + 4]),
            "wv": np.ascontiguousarray(wv_full[:, :, 512 * hf : 512 * hf + 512]),
        })
    return shared, halves


def _prep_lm(lm_W, lm_b):
    """Quantize + pack the lm_head for the fp8 DoubleRow phase 4.

    Returns per-vocab-half (lmw8 [NVT,128,4,2,128] u8, lmbt [128,NVT] f32)."""
    import ml_dtypes

    outs = []
    for v in range(2):
        Wsh = np.asarray(lm_W[:, v * VSH : (v + 1) * VSH], dtype=np.float32)
        W8 = np.clip(Wsh * FP8_SCALE, -240.0, 240.0).astype(ml_dtypes.float8_e4m3)
        # [E, VSH] -> [kp, s, p, vt, m] -> [vt, p, kp, s, m]
        t = W8.reshape(4, 2, 128, NVT, 128)
        lmw8 = (
            np.ascontiguousarray(t.transpose(3, 2, 0, 1, 4))
            .view(np.uint8)
            .reshape(NVT // 5, 5, 128, 4, 2, 128)
        )
        bsh = np.asarray(lm_b[v * VSH : (v + 1) * VSH], dtype=np.float32)
        lmbt = np.ascontiguousarray(bsh.reshape(NVT, 128).T)
        outs.append((lmw8, lmbt))
    return outs


def kernel(idx, tok_emb, pos_emb, Wq, Wk, Wv, lm_W, lm_b):
    from concourse.bass_utils import run_bass_kernel_spmd

    if "nc" not in _cache:
        _cache["nc"] = _build_nc()
    nc = _cache["nc"]

    idx = np.asarray(idx)
    shared, head_halves = _prep_shared(
        np.asarray(tok_emb), np.asarray(pos_emb), np.asarray(Wq), np.asarray(Wk), np.asarray(Wv)
    )
    lm_parts = _prep_lm(np.asarray(lm_W), np.asarray(lm_b))

    in_maps = []
    for c in range(8):
        b, v = c // 2, c % 2
        m = dict(shared)
        m.update(head_halves[v])
        m["idx"] = np.ascontiguousarray(
            idx[b].astype(np.int32).reshape(NT, 128).T
        )
        m["lmw8"], m["lmbt"] = lm_parts[v]
        in_maps.append(m)

    res = run_bass_kernel_spmd(nc, in_maps, core_ids=list(range(8)))
    logits = np.empty((B, T, VOCAB), np.float32)
    for c in range(8):
        b, v = c // 2, c % 2
        lg = res.results[c]["logits"].reshape(VSH, T)
        logits[b, :, v * VSH : (v + 1) * VSH] = lg.T
    return logits



# revision 16
# speedup vs baseline: 1.2781x; 1.0228x over previous
"""Trainium2 Bass kernel for a single-layer bigram language model
(embed + 16-head causal attention + vocab lm_head).

Sharding: 8 cores = 4 batches x 2 vocab halves. Core c handles batch
c//2, vocab columns [(c%2)*16000, ...), and computes attention for 8 of
the 16 heads (even cores: heads 0-7, odd: 8-15 -- pure SPMD, the head
assignment comes from which Wq/Wk/Wv slices each core receives). The
fp8 attention output (+ its token-tile-0 residual) is exchanged between
the two cores of a batch pair with two small AllGathers (pair replica
groups), pipelined against the second half of the attention loop.

The lm_head runs in fp8 (e4m3) DoubleRow perf mode (two contraction
rows per PE column-cycle => 2x bf16 throughput). Both operands are
scaled by 2^11 before quantization; the product descale 2^-22 plus the
bias add are folded into one Activation op per output tile. Accuracy:
the max-abs error of full-fp8 logits is driven entirely by the first
~128 tokens (softmax over few positions => large attention-out rows).
Fix: an fp8 *residual* of the attention out for token tile 0 (same 2^11
scale, so it accumulates into the same PSUM group) adds a second-order
correction only where needed. Measured on HW: rel err 1.63e-2 (gate
2e-2); full-fp8 without the correction would be 2.2e-2.

Phase-4 layout is [vocab_partition, token_free]:
  - bias becomes per-partition => fused into the ACT descale op
  - logits DMA out as [VSH, T]; the host transposes when assembling.
Phase-4 DMAs are grouped over VG=5 vocab tiles (HWDGE charges a fixed
~625ns per DMA instruction) and the entire 16MB fp8 lm_W prefetches
during the attention phase (lwp bufs=25 covers all 25 groups).

All other matmuls run as float32r (fp32 data, PE truncates to fp22)
which runs at full bf16 speed for moving dims >= 256, with fp32 PSUM
accumulation. Attention epilogue work is spread across DVE (normalize,
residual), ACT (exp, q/k PSUM->SBUF copies), and Pool/GPSIMD (half the
causal-mask multiplies) to keep DVE off the critical path.

Layout strategy: everything downstream of the embedding is kept
transposed ([feature, token]) so all matmul contractions have their K
dim on partitions:
  xT[e, t]  = (tok_emb[idx] + pos_emb).T        (PE-transposed after gather)
  qT/kT     = Wq_pair.T @ xT  (pairs of heads stacked on partitions 0:64 / 64:128)
  scoresT   = kT_h.T-slices @ qT_h  -> [s, t] tiles, causally trimmed;
              exp on ACT; boundary masks on DVE/Pool
  pv        = [v_h | 1].T @ e       -> rows 0:64 = unnormalized outT, row 64 = denom
  outT8     = fp8(pv * (2^11/denom broadcast))  (odd heads DMA-shifted to partitions 64:128)
  logitsT   = (outT8 [+ outT8r]).T @ lmW8 * 2^-22 + lm_b   (vocab on partitions)

TimelineSim estimate: ~381us/core (baseline f32r version: ~718us).
"""

import sys

if "/opt/trn_rl_repo" not in sys.path:
    sys.path.insert(0, "/opt/trn_rl_repo")

import numpy as np

VOCAB = 32000
E = 1024
T = 1024
H = 16
HS = 64
B = 4
VSH = VOCAB // 2  # per-core vocab shard
NE = E // 128  # 8 e-tiles
NT = T // 128  # 8 t-tiles
NVT = VSH // 128  # 125 vocab tiles (128 wide, on partitions in phase 4)
FP8_SCALE = 2.0**11  # scale for both fp8 operands of the lm_head
DESCALE = float(2.0 ** (-22))

_cache = {}


def _build_nc(tune=None):
    tune = dict(tune or {})

    def tn(k, d):
        return tune.get(k, d)

    import concourse.bass as bass
    import concourse.bacc as bacc
    import concourse.tile as tile
    from concourse import mybir
    from contextlib import ExitStack

    f32 = mybir.dt.float32
    f32r = mybir.dt.float32r
    f8 = mybir.dt.float8e4
    u8 = mybir.dt.uint8
    i32 = mybir.dt.int32

    nc = bacc.Bacc("TRN2", target_bir_lowering=False, debug=False)

    idx_d = nc.dram_tensor("idx", [128, NT], i32, kind="ExternalInput").ap()
    tok_d = nc.dram_tensor("tok", [VOCAB, E], f32, kind="ExternalInput").ap()
    post_d = nc.dram_tensor("post", [NE, 128, T], f32, kind="ExternalInput").ap()
    HSPL = tn("hsplit", 1)
    NPR = 4 if HSPL else 8  # head-pairs computed locally
    NHL = 2 * NPR  # local heads
    wq_d = nc.dram_tensor("wq", [NPR, NE, 128, 128], f32, kind="ExternalInput").ap()
    wk_d = nc.dram_tensor("wk", [NPR, NE, 128, 128], f32, kind="ExternalInput").ap()
    wv_d = nc.dram_tensor("wv", [NE, 128, 64 * NHL], f32, kind="ExternalInput").ap()
    msk_d = nc.dram_tensor("msk", [4, 128, 512], f32, kind="ExternalInput").ap()
    idn_d = nc.dram_tensor("idn", [128, 128], f32, kind="ExternalInput").ap()
    # fp8 lm_head weights, packed [vt, p, kp, s, m] and grouped by VG
    # vocab tiles per DMA: W8[kp*256 + s*128 + p, vt*128 + m]
    VG = tn("vg", 5)
    NVG = NVT // VG
    lmw8_d = nc.dram_tensor(
        "lmw8", [NVG, VG, 128, 4, 2, 128], u8, kind="ExternalInput"
    ).ap()
    # bias pre-transposed to [p, vt]
    lmbt_d = nc.dram_tensor("lmbt", [128, NVT], f32, kind="ExternalInput").ap()
    # logits transposed: [vocab_shard, tokens]; host transposes back
    out_d = nc.dram_tensor("logits", [NVG, VG, 128, T], f32, kind="ExternalOutput").ap()

    EXP = mybir.ActivationFunctionType.Exp
    IDENT = mybir.ActivationFunctionType.Identity

    with tile.TileContext(nc) as tc, ExitStack() as ctx:
        const = ctx.enter_context(tc.tile_pool(name="const", bufs=1))
        persist = ctx.enter_context(tc.tile_pool(name="persist", bufs=1))

        idxt = const.tile([128, NT], i32)
        nc.sync.dma_start(idxt[:], idx_d[:])
        idn = const.tile([128, 128], f32)
        nc.sync.dma_start(idn[:], idn_d[:])
        msk = const.tile([128, 4, 512], f32)
        ones = const.tile([128, 1], f32)
        nc.vector.memset(ones[:], 1.0)
        # the out-normalization broadcast carries the fp8 up-scale for free
        sc64r = const.tile([128, 64], f32r)
        scv = const.tile([128, 1], f32)
        nc.vector.memset(scv[:], FP8_SCALE)
        nc.vector.tensor_copy(sc64r[:], scv[:].to_broadcast([128, 64]))

        outT8 = persist.tile([128, NE, T], f8)
        outT8r = persist.tile([128, NE, 128], f8)  # token-tile-0 fp8 residual
        # attention-lifetime tensors live in their own pool so the SBUF is
        # reclaimed before phase 4 (frees room for the full lm_W8 prefetch)
        attnp_cm = tc.tile_pool(name="attn", bufs=1)
        attnp = attnp_cm.__enter__()
        xT = attnp.tile([128, NE, T], f32r)
        outT0f = attnp.tile([128, NPR, 128], f32)  # token-tile-0 f32 scratch
        vaug = attnp.tile([128, NT, NHL, HS + 1], f32r)

        # ---------- Phase 1+2 interleaved: per token tile, gather ->
        # transpose+positional add -> V matmul. Interleaving keeps the
        # (in-order) PE busy with V work during the gather-wait gaps.
        with (
            tc.tile_pool(name="ph1", bufs=tn("ph1", 4)) as gp,
            tc.tile_pool(name="postp", bufs=1) as postp,
            tc.tile_pool(name="wvp", bufs=1) as wvp,
            tc.tile_pool(name="pst", bufs=tn("pst", 5), space="PSUM") as pst,
            tc.tile_pool(name="pv2", bufs=tn("pv2", 3), space="PSUM") as pv2,
        ):
            post_sb = postp.tile([128, NE, T], f32)
            wv_sb = wvp.tile([128, NE, 64 * NHL], f32r)
            xgs = {}

            def issue_gather(tt):
                xg = gp.tile([128, E], f32, tag="xg")
                nc.gpsimd.indirect_dma_start(
                    out=xg[:],
                    out_offset=None,
                    in_=tok_d[:],
                    in_offset=bass.IndirectOffsetOnAxis(
                        ap=idxt[:, tt : tt + 1], axis=0
                    ),
                )
                return xg

            for tt in range(NT if tn("pregather", 1) else 1):
                xgs[tt] = issue_gather(tt)
            for k in range(NE):
                (nc.scalar if tn("post_act", 0) else nc.sync).dma_start(
                    post_sb[:, k, :], post_d[k]
                )
            for k in range(NE):
                (nc.scalar if tn("wv_act", 0) else nc.sync).dma_start(
                    wv_sb[:, k, :], wv_d[k].bitcast(f32r)
                )
            for tt in range(NT):
                xg = xgs.get(tt) or issue_gather(tt)
                for k in range(NE):
                    ps = pst.tile([128, 128], f32, tag="tp")
                    nc.tensor.transpose(
                        out=ps[:], in_=xg[:, k * 128 : (k + 1) * 128], identity=idn[:]
                    )
                    nc.vector.tensor_add(
                        xT[:, k, tt * 128 : (tt + 1) * 128],
                        ps[:],
                        post_sb[:, k, tt * 128 : (tt + 1) * 128],
                    )
                st = tt
                for ns in range(NHL // 8):
                    ps = pv2.tile([128, 512], f32, tag="vps")
                    for k in range(NE):
                        nc.tensor.matmul(
                            ps[:],
                            xT[:, k, st * 128 : (st + 1) * 128],
                            wv_sb[:, k, ns * 512 : (ns + 1) * 512],
                            start=(k == 0),
                            stop=(k == NE - 1),
                        )
                    for hh in range(8):
                        h = ns * 8 + hh
                        if tn("vaug_act", 1):
                            nc.scalar.activation(
                                vaug[:, st, h, 0:HS], ps[:, hh * 64 : (hh + 1) * 64], IDENT
                            )
                        else:
                            nc.vector.tensor_copy(
                                vaug[:, st, h, 0:HS], ps[:, hh * 64 : (hh + 1) * 64]
                            )
                nc.vector.tensor_copy(
                    vaug[:, st, :, HS : HS + 1], ones[:].to_broadcast([128, NHL, 1])
                )

        # ---------- Phase 3: attention, one head pair at a time
        nc.sync.dma_start(msk[:], msk_d.rearrange("k p n -> p k n"))
        with (
            tc.tile_pool(name="wqk", bufs=tn("wqk", 2)) as wqk,
            tc.tile_pool(name="qkp", bufs=tn("qkp", 2)) as qkp,
            tc.tile_pool(name="ep", bufs=tn("ep", 9)) as ep,
            tc.tile_pool(name="rp", bufs=tn("rp", 4)) as rp,
            tc.tile_pool(name="sp", bufs=tn("sp", 4)) as sp,
            tc.tile_pool(name="pqk", bufs=tn("pqk", 2), space="PSUM") as pqk,
            tc.tile_pool(name="psc", bufs=tn("psc", 3), space="PSUM") as psc,
            tc.tile_pool(name="ppv", bufs=tn("ppv", 2), space="PSUM") as ppv,
            tc.tile_pool(name="pbc", bufs=tn("pbc", 1), space="PSUM") as pbc,
        ):
            ccp = ctx.enter_context(
                tc.tile_pool(name="ccp", bufs=1, space="DRAM")
            )
            PW = T + 128  # packed width: outT8 row + residual row
            for pr in range(NPR):
                qT = qkp.tile([128, T], f32r, tag="qT")
                kT = qkp.tile([128, T], f32r, tag="kT")
                for wd, dst, wtag in ((wq_d, qT, "wq"), (wk_d, kT, "wk")):
                    w_sb = wqk.tile([128, NE, 128], f32r, tag=wtag)
                    nc.sync.dma_start(w_sb[:], wd[pr].rearrange("k p n -> p k n").bitcast(f32r))
                    for ts2 in range(2):
                        ps = pqk.tile([128, 512], f32, tag="qkps")
                        for k in range(NE):
                            nc.tensor.matmul(
                                ps[:],
                                w_sb[:, k, :],
                                xT[:, k, ts2 * 512 : (ts2 + 1) * 512],
                                start=(k == 0),
                                stop=(k == NE - 1),
                            )
                        if tn("qk_act", 1):
                            nc.scalar.activation(
                                dst[:, ts2 * 512 : (ts2 + 1) * 512], ps[:], IDENT
                            )
                        else:
                            nc.vector.tensor_copy(dst[:, ts2 * 512 : (ts2 + 1) * 512], ps[:])
                for sub in range(2):
                    h = 2 * pr + sub
                    q_s = qT[sub * 64 : (sub + 1) * 64, :]
                    k_s = kT[sub * 64 : (sub + 1) * 64, :]
                    for j in range(2):
                        smax = 4 * j + 3
                        e_tiles = []
                        for i in range(smax + 1):
                            ko = i - 4 * j
                            # causal trim: columns below the diagonal block
                            # are fully masked; skip them on PE/ACT/DVE
                            off = 128 * ko if (ko > 0 and tn("trim", 1)) else 0
                            ps = psc.tile([128, 512], f32, tag="sc")
                            nc.tensor.matmul(
                                ps[:, off:512],
                                k_s[:, i * 128 : (i + 1) * 128],
                                q_s[:, j * 512 + off : (j + 1) * 512],
                                start=True,
                                stop=True,
                            )
                            et = ep.tile([128, 512], f32r, tag="e")
                            nc.scalar.activation(et[:, off:512], ps[:, off:512], EXP)
                            if ko >= 0:
                                # alternate mask engine DVE/Pool to balance
                                meng = nc.gpsimd if (i % 2 == tn("mpar", 1)) else nc.vector
                                meng.tensor_mul(
                                    et[:, off:512], et[:, off:512], msk[:, ko, off:512]
                                )
                            e_tiles.append((off, et))
                        pv = ppv.tile([128, 512], f32, tag="pv")
                        for n, (off, et) in enumerate(e_tiles):
                            nc.tensor.matmul(
                                pv[0:65, off:512],
                                vaug[:, n, h, :],
                                et[:, off:512],
                                start=(n == 0),
                                stop=(n == smax),
                                skip_group_check=True,
                            )
                        rc = rp.tile([128, 512], f32, tag="rc")
                        # custom-DVE reciprocal can't read PSUM on HW (CoreSim
                        # diverges); stage the denominator row into SBUF first
                        nc.vector.reciprocal(rc[64:65, :], pv[64:65, :])
                        # partition-broadcast row 64 -> rows 0:64 via a K=1
                        # matmul (sc64[64] outer recip_row); DMA-based SBUF
                        # broadcasts race on HW. sc64r = 2^11 folds in the
                        # fp8 up-scale.
                        rcr = rp.tile([128, 512], f32r, tag="rcr")
                        nc.vector.tensor_copy(rcr[64:65, :], rc[64:65, :])
                        pb = pbc.tile([128, 512], f32, tag="pb")
                        nc.tensor.matmul(
                            pb[0:64, :],
                            sc64r[64:65, :],
                            rcr[64:65, :],
                            start=True,
                            stop=True,
                        )
                        rcb = rp.tile([128, 512], f32, tag="rcb")
                        (nc.gpsimd if tn("rcb_pool", 0) else nc.vector).tensor_copy(
                            rcb[0:64, :], pb[0:64, :]
                        )
                        jsl = slice(j * 512, (j + 1) * 512)
                        if sub == 0:
                            nc.vector.tensor_mul(
                                outT8[0:64, pr, jsl], pv[0:64, :], rcb[0:64, :]
                            )
                            if j == 0:
                                nc.vector.tensor_mul(
                                    outT0f[0:64, pr, :], pv[0:64, 0:128], rcb[0:64, 0:128]
                                )
                                rsb = sp.tile([128, 128], f32, tag="rsb")
                                nc.vector.tensor_sub(
                                    rsb[0:64, :], outT0f[0:64, pr, :], outT8[0:64, pr, 0:128]
                                )
                                nc.vector.tensor_copy(outT8r[0:64, pr, :], rsb[0:64, :])
                        else:
                            stg = sp.tile([128, 512], f8, tag="stg")
                            nc.vector.tensor_mul(stg[0:64, :], pv[0:64, :], rcb[0:64, :])
                            nc.sync.dma_start(outT8[64:128, pr, jsl], stg[0:64, :])
                            if j == 0:
                                nc.vector.tensor_mul(
                                    outT0f[0:64, pr, :], pv[0:64, 0:128], rcb[0:64, 0:128]
                                )
                                rsb = sp.tile([128, 128], f32, tag="rsb")
                                nc.vector.tensor_sub(
                                    rsb[0:64, :], outT0f[0:64, pr, :], stg[0:64, 0:128]
                                )
                                rs8 = sp.tile([128, 128], f8, tag="rs8")
                                nc.vector.tensor_copy(rs8[0:64, :], rsb[0:64, :])
                                nc.sync.dma_start(outT8r[64:128, pr, :], rs8[0:64, :])
                CCN = tn("ccn", 2)  # head-pairs per exchange collective
                if CCN == 3:
                    # asymmetric: bulk exchange after pr2, small tail after pr3
                    fire = {2: (0, 0, 3), 3: (1, 3, 4)}.get(pr)
                else:
                    fire = (
                        (pr // CCN, CCN * (pr // CCN), CCN * (pr // CCN) + CCN)
                        if pr % CCN == CCN - 1
                        else None
                    )
                if HSPL and fire is not None:
                    # local pairs done: AllGather with the partner core.
                    # Pack outT8 + residual into one buffer.
                    half, klo, khi = fire
                    CCW = khi - klo
                    ksl = slice(klo, khi)
                    cin = ccp.tile([128, CCW, PW], f8, space="DRAM", tag=f"cin{half}")
                    cout = ccp.tile(
                        [2, 128, CCW, PW], f8, space="DRAM", tag=f"cout{half}"
                    )
                    nc.sync.dma_start(cin[:, :, 0:T], outT8[:, ksl, :])
                    nc.sync.dma_start(cin[:, :, T:PW], outT8r[:, ksl, :])
                    nc.gpsimd.collective_compute(
                        kind="AllGather",
                        op=mybir.AluOpType.bypass,
                        replica_groups=[[0, 1], [2, 3], [4, 5], [6, 7]],
                        ins=[cin[:].bitcast(u8)],
                        outs=[cout[:].bitcast(u8)],
                    )
                    # unpack both ranks: SPMD means local heads may sit in
                    # the other global half (odd cores own heads 8..15)
                    for r in range(2):
                        gk = slice(4 * r + klo, 4 * r + khi)
                        nc.sync.dma_start(outT8[:, gk, :], cout[r, :, :, 0:T])
                        nc.sync.dma_start(outT8r[:, gk, :], cout[r, :, :, T:PW])

        attnp_cm.__exit__(None, None, None)

        # ---------- Phase 4: lm head, fp8 DoubleRow, [vocab, token] layout.
        # DMAs are grouped over VG vocab tiles: HWDGE charges a fixed
        # ~625ns per DMA instruction, so fewer/bigger transfers matter.
        with (
            tc.tile_pool(name="lwp", bufs=tn("lwp", 25)) as lwp,
            tc.tile_pool(name="bp", bufs=1) as bp,
            tc.tile_pool(name="ogp", bufs=tn("ogp", 3)) as ogp,
            tc.tile_pool(name="plm", bufs=tn("plm", 8), space="PSUM") as plm,
        ):
            lmb_sb = bp.tile([128, NVT], f32)
            nc.sync.dma_start(lmb_sb[:], lmbt_d[:])
            for vg in range(NVG):
                lw = lwp.tile([128, VG, 4, 2, 128], f8, tag="lw")
                # issue lm_W prefetch on the ACT DGE queue so it is
                # not FIFO-ordered behind the og output stores on SP's queue
                (nc.scalar if tn("lw_act", 0) else nc.sync).dma_start(
                    lw[:], lmw8_d[vg].rearrange("g p a b c -> p g a b c").bitcast(f8)
                )
                og = ogp.tile([128, VG, T], f32, tag="og")
                for g in range(VG):
                    vt = vg * VG + g
                    for tt in range(2):
                        ps = plm.tile([128, 512], f32, tag="lm")
                        tsl = slice(tt * 512, (tt + 1) * 512)
                        for kp in range(4):
                            nc.tensor.matmul(
                                ps[:],
                                lw[:, g, kp],
                                outT8[:, 2 * kp : 2 * kp + 2, tsl],
                                start=(kp == 0),
                                stop=(kp == 3 and tt == 1),
                                perf_mode=mybir.MatmulPerfMode.DoubleRow,
                            )
                        if tt == 0:
                            # token-tile-0 residual correction, same PSUM group
                            for kp in range(4):
                                nc.tensor.matmul(
                                    ps[:, 0:128],
                                    lw[:, g, kp],
                                    outT8r[:, 2 * kp : 2 * kp + 2, :],
                                    start=False,
                                    stop=(kp == 3),
                                    perf_mode=mybir.MatmulPerfMode.DoubleRow,
                                    skip_group_check=True,
                                )
                        if tn("og_dve", 0) and (2 * g + tt) % 2 == 1:
                            nc.vector.scalar_tensor_tensor(
                                og[:, g, tsl],
                                ps[:],
                                DESCALE,
                                lmb_sb[:, vt : vt + 1].to_broadcast([128, 512]),
                                mybir.AluOpType.mult,
                                mybir.AluOpType.add,
                            )
                        else:
                            nc.scalar.activation(
                                og[:, g, tsl],
                                ps[:],
                                IDENT,
                                bias=lmb_sb[:, vt : vt + 1],
                                scale=DESCALE,
                            )
                if tn("og_split", 0):
                    for g in range(VG):
                        nc.sync.dma_start(out_d[vg, g], og[:, g, :])
                else:
                    nc.sync.dma_start(
                        out_d[vg].rearrange("g p t -> p g t"), og[:]
                    )

    nc.compile()
    return nc


def _prep_shared(tok_emb, pos_emb, Wq, Wk, Wv):
    f = np.float32
    tok = np.ascontiguousarray(tok_emb, dtype=f)
    post = np.ascontiguousarray(pos_emb.T.reshape(NE, 128, T), dtype=f)

    def pair_stack(W):
        out = np.empty((8, NE, 128, 128), dtype=f)
        for p in range(8):
            pairw = np.concatenate([W[2 * p], W[2 * p + 1]], axis=1)  # [E, 128]
            out[p] = pairw.reshape(NE, 128, 128)
        return out

    wq_full = pair_stack(np.asarray(Wq, dtype=f))
    wk_full = pair_stack(np.asarray(Wk, dtype=f))
    wv_full = np.ascontiguousarray(
        np.asarray(Wv, dtype=f).transpose(1, 0, 2).reshape(E, H * HS).reshape(NE, 128, E)
    )
    # causal masks: msk[ko][p, c] = 1.0 if p + 128*ko <= c else 0.0
    p_idx = np.arange(128)[:, None]
    c_idx = np.arange(512)[None, :]
    msk = np.stack(
        [(p_idx + 128 * ko <= c_idx).astype(f) for ko in range(4)]
    )
    idn = np.eye(128, dtype=f)
    shared = {"tok": tok, "post": post, "msk": msk, "idn": idn}
    halves = []
    for hf in range(2):
        halves.append({
            "wq": np.ascontiguousarray(wq_full[4 * hf : 4 * hf + 4]),
            "wk": np.ascontiguousarray(wk_full[4 * hf : 4 * hf + 4]),
            "wv": np.ascontiguousarray(wv_full[:, :, 512 * hf : 512 * hf + 512]),
        })
    return shared, halves


def _prep_lm(lm_W, lm_b):
    """Quantize + pack the lm_head for the fp8 DoubleRow phase 4.

    Returns per-vocab-half (lmw8 [NVT,128,4,2,128] u8, lmbt [128,NVT] f32)."""
    import ml_dtypes

    outs = []
    for v in range(2):
        Wsh = np.asarray(lm_W[:, v * VSH : (v + 1) * VSH], dtype=np.float32)
        W8 = np.clip(Wsh * FP8_SCALE, -240.0, 240.0).astype(ml_dtypes.float8_e4m3)
        # [E, VSH] -> [kp, s, p, vt, m] -> [vt, p, kp, s, m]
        t = W8.reshape(4, 2, 128, NVT, 128)
        lmw8 = (
            np.ascontiguousarray(t.transpose(3, 2, 0, 1, 4))
            .view(np.uint8)
            .reshape(NVT // 5, 5, 128, 4, 2, 128)
        )
        bsh = np.asarray(lm_b[v * VSH : (v + 1) * VSH], dtype=np.float32)
        lmbt = np.ascontiguousarray(bsh.reshape(NVT, 128).T)
        outs.append((lmw8, lmbt))
    return outs


def kernel(idx, tok_emb, pos_emb, Wq, Wk, Wv, lm_W, lm_b):
    from concourse.bass_utils import run_bass_kernel_spmd

    if "nc" not in _cache:
        _cache["nc"] = _build_nc()
    nc = _cache["nc"]

    idx = np.asarray(idx)
    shared, head_halves = _prep_shared(
        np.asarray(tok_emb), np.asarray(pos_emb), np.asarray(Wq), np.asarray(Wk), np.asarray(Wv)
    )
    lm_parts = _prep_lm(np.asarray(lm_W), np.asarray(lm_b))

    in_maps = []
    for c in range(8):
        b, v = c // 2, c % 2
        m = dict(shared)
        m.update(head_halves[v])
        m["idx"] = np.ascontiguousarray(
            idx[b].astype(np.int32).reshape(NT, 128).T
        )
        m["lmw8"], m["lmbt"] = lm_parts[v]
        in_maps.append(m)

    res = run_bass_kernel_spmd(nc, in_maps, core_ids=list(range(8)))
    logits = np.empty((B, T, VOCAB), np.float32)
    for c in range(8):
        b, v = c // 2, c % 2
        lg = res.results[c]["logits"].reshape(VSH, T)
        logits[b, :, v * VSH : (v + 1) * VSH] = lg.T
    return logits

